# revision 18
# baseline (speedup 1.0000x reference)
"""MoE-routed group-norm kernel for Trainium2 (Bass/Tile), 8-core SPMD.

Problem (hardcoded shapes):
  x: [64, 512, 32, 32] f32
  experts_weight/bias: [8, 512], shared_weight/bias: [512]
  router_w: [8, 512], router_b: [8]

  flat = x.mean((2,3)); logits = flat @ router_w.T + router_b
  prob = softmax(logits); top-2 -> coeff = vals / sum(vals)
  fused_w = sum_k coeff_k * experts_weight[idx_k] + shared_weight (bias likewise)
  group-norm over G=32 groups of 16 channels, then y = x_norm * fused_w + fused_b

Strategy: data-parallel over batch, 8 samples per core.

HBM-traffic decisions (this problem is memory-bound):
  * y is stored as bf16 and widened to f32 on the host: 24 MiB/core
    instead of 32 MiB. x is cast f32->bf16 during the load DMA (SWDGE), so
    HBM reads stay f32 but all on-chip passes run 16-bit.
  * channel->partition map is c = 4p + t (NOT c = 128t + p): each
    partition's slice of a sample is CONTIGUOUS in DRAM (16 KiB in, 8 KiB
    out), which gives large DMA descriptors on both directions.
  * all 8 x tiles stay resident in SBUF and every load is pre-issued up
    front on the gpsimd (SWDGE) queue, so the load stream never waits on
    buffer recycling; stores issue from the sync HWDGE ring as each
    sample's pass2 completes.

With c = 4p + t, group(c) = c//16 = p//4: a group never spans chunks, so
group stats are per-partition-quad only:
  s1 per (channel) = one DVE X-reduce per sample ([P,4,1024] -> [P,4])
  s2 per partition = one ACT Square+accum_out per sample (sums all 4096)
  group sums = one PE matmul vs gmask32 [128,32] (pre-scaled by 1/16384)
  pass2 y = A*x + B: chunks 0,1 DVE tensor_scalar (bf16, fast mode),
    chunks 2,3 ACT Identity (scale/bias APs)

Routing is the known-good [2,E] pair-batched form: top-1 exp is exactly
1.0 so the softmax denominator cancels in coeff = vals/sum(vals); ACT's
table stays pinned to exp_and_others. Since sum(coeff)=1, shared
weight/bias are folded into the expert tables on the host. rstd uses the
bit-trick seed + one Newton step on DVE (~0.2% rel err, fine at bf16
output precision). Expert mixing / broadcasts are small PE matmuls in
bf16 (fp32 PE matmuls cost 2 instructions each; routing-relevant s1/logits
stay f32 so top-2 selection matches the reference bit-for-bit). PSUM and
ACT-written tiles use static per-pair regions.
"""

import numpy as np

import concourse.bacc as bacc
import concourse.bass as bass
import concourse.tile as tile
from concourse import mybir
from concourse.bass_utils import run_bass_kernel_spmd

F32 = mybir.dt.float32
BF16 = mybir.dt.bfloat16
I32 = mybir.dt.int32
ALU = mybir.AluOpType
ACTF = mybir.ActivationFunctionType
AXX = mybir.AxisListType.X

P = 128            # SBUF partitions
B, C, HWD = 64, 512, 1024
E, G = 8, 32
EPS = 1e-5
NCORES = 8
BPC = B // NCORES  # samples per core
NCH = C // P       # 4 channel chunks per sample (t axis; c = 4p + t)
CPG = C // G       # 16 channels per group
PAIR = 2
RSQRT_MAGIC = 0x5F3759DF
GSCALE = 1.0 / (CPG * HWD)

# cA (f32) layout [128, 74]:
#   0:32  routerT  (ca[p, 8t+e] = router_w[e, 4p+t] / 1024)
#   32:64 gmask32  (ca[p, 32+g] = (p//4 == g) / 16384)
#   64:72 rb2 (rows 0:2) | 72:74 ident2 (rows 0:2)
CA_W = 74
# cB (bf16) layout [32, 1152]:
#   0:128 bmask32 (cb[g, p] = (p//4 == g))
#   rows 0:8 only -- 128:640 ew' (cb[e, 128+128t+p] = ew'[e, 4p+t]) | 640:1152 eb'
CB_W = 1152


def build(n_b: int = BPC) -> bass.Bass:
    assert n_b % PAIR == 0
    npair = n_b // PAIR
    nc = bacc.Bacc()
    x_d = nc.declare_dram_parameter("x", [n_b, C, HWD], F32, isOutput=False)
    ca_d = nc.declare_dram_parameter("ca", [P, CA_W], F32, isOutput=False)
    cb_d = nc.declare_dram_parameter("cb", [G, CB_W], BF16, isOutput=False)
    y_d = nc.declare_dram_parameter("y", [n_b, C, HWD], BF16, isOutput=True)

    with tile.TileContext(nc) as tc:
        with (
            tc.tile_pool(name="consts", bufs=1) as consts,
            tc.tile_pool(name="xp", bufs=n_b) as xp,
            tc.tile_pool(name="yp", bufs=n_b) as yp,
            tc.tile_pool(name="scr", bufs=2) as scrp,
            tc.tile_pool(name="statp", bufs=4) as statp,
            tc.tile_pool(name="tinyp", bufs=4) as tinyp,
            tc.tile_pool(name="ps_static", bufs=1, space="PSUM") as pstat,
        ):
            # consts staged through a DVE copy so PE inputs have DVE provenance
            ca_st = consts.tile([P, CA_W], F32)
            nc.sync.dma_start(out=ca_st, in_=ca_d[:, :])
            cb_st = consts.tile([G, CB_W], BF16)
            nc.sync.dma_start(out=cb_st, in_=cb_d[:, :])
            ca = consts.tile([P, CA_W], F32)
            nc.vector.tensor_copy(ca, ca_st)
            cb = consts.tile([G, CB_W], BF16)
            nc.vector.tensor_copy(cb, cb_st)
            magic32 = consts.tile([G, PAIR], F32)
            nc.vector.memset(magic32[:, :].bitcast(I32), RSQRT_MAGIC)
            one32 = consts.tile([G, PAIR], F32)
            nc.vector.memset(one32[:, :].bitcast(I32), 1)

            gmask = ca[:, 32:64]
            rb2 = ca[0:PAIR, 64:72]
            ident2 = ca[0:PAIR, 72:74]
            bmask = cb[:, 0:P]

            # all 8 x tiles resident; every load pre-issued on the SWDGE
            # queue (f32 -> bf16 cast during DMA), two halves per sample
            xts_all = []
            for b in range(n_b):
                x_t = xp.tile([P, NCH, HWD], BF16, tag="x")
                xts_all.append(x_t)
                xv = x_d[b].rearrange("(p t) f -> p t f", p=P)
                if b == 0:
                    for j4 in range(NCH):
                        nc.gpsimd.dma_start(
                            out=x_t[:, j4 : j4 + 1, :], in_=xv[:, j4 : j4 + 1, :]
                        )
                else:
                    nc.gpsimd.dma_start(out=x_t[:, 0:2, :], in_=xv[:, 0:2, :])
                    nc.gpsimd.dma_start(out=x_t[:, 2:4, :], in_=xv[:, 2:4, :])

            # static per-pair PSUM regions (never reused -> no PSUM WAW deps)
            ps_sm = pstat.tile([P, npair, 20], F32, tag="sm")
            ps_fu = pstat.tile([P, npair, 2, NCH, PAIR], F32, tag="fu")
            ps_bc = pstat.tile([P, npair, PAIR, 2], F32, tag="bc")
            erow_all = consts.tile([PAIR, npair, E], F32)

            def stage1(ip):
                xts = [xts_all[ip * PAIR], xts_all[ip * PAIR + 1]]
                # s12 [P, 10]: cols 0:8 s1 per (bb, t), cols 8:10 s2 per bb
                s12 = statp.tile([P, 10], F32, tag="s12")
                s1v = s12[:, 0:8].rearrange("p (b t) -> p b t", t=NCH)
                lg_ps = ps_sm[0:PAIR, ip, 10:18]      # logits [2, 8]

                for bb in range(PAIR):
                    # two 2-chunk reduces: smaller granules let the
                    # scheduler slot chain ops between them
                    nc.vector.reduce_sum(
                        s1v[:, bb, 0:2], xts[bb][:, 0:2, :], axis=AXX
                    )
                    nc.vector.reduce_sum(
                        s1v[:, bb, 2:4], xts[bb][:, 2:4, :], axis=AXX
                    )
                    sq = scrp.tile([P, NCH, HWD], BF16, tag="sq")
                    nc.scalar.activation(
                        sq,
                        xts[bb][:, :, :],
                        ACTF.Square,
                        bias=0.0,
                        scale=1.0,
                        accum_out=s12[:, 8 + bb : 9 + bb],
                    )

                # logits[s, e] = sum_c s1[c, s]/1024 * router_w[e, c]
                for t in range(NCH):
                    nc.tensor.matmul(
                        lg_ps,
                        s1v[:, :, t],
                        ca[:, t * 8 : (t + 1) * 8],
                        start=(t == 0),
                        stop=(t == NCH - 1),
                    )
                # group sums of (s1 | s2), pre-scaled by 1/16384 via gmask
                gs_ps = ps_sm[0:G, ip, 0:10]
                nc.tensor.matmul(gs_ps, gmask, s12[:, :])
                return xts

            def stage2(ip, xts):
                gs_ps = ps_sm[0:G, ip, 0:10]
                lg_ps = ps_sm[0:PAIR, ip, 10:18]
                ct_ps = ps_sm[0:E, ip, 18:20]

                # routing, pair-batched in [2, E] partition layout
                lrow = tinyp.tile([PAIR, E], F32, tag="lrow")
                nc.vector.tensor_tensor(lrow, lg_ps, rb2, ALU.add)
                nmax = tinyp.tile([PAIR, 1], F32, tag="nmax")
                nc.vector.reduce_max(nmax, lrow, axis=AXX, negate=True)
                erow = erow_all[:, ip, :]
                nc.scalar.activation(erow, lrow, ACTF.Exp, bias=nmax, scale=1.0)
                qrow = tinyp.tile([PAIR, E], F32, tag="qrow")
                nc.vector.scalar_tensor_tensor(
                    qrow, erow, 1.0, erow, op0=ALU.is_lt, op1=ALU.mult
                )
                m2 = tinyp.tile([PAIR, 1], F32, tag="m2")
                nc.vector.reduce_max(m2, qrow, axis=AXX)
                gate = tinyp.tile([PAIR, E], F32, tag="gate")
                nc.vector.scalar_tensor_tensor(
                    gate, erow, m2[:, 0:1], erow, op0=ALU.is_ge, op1=ALU.mult
                )
                den = tinyp.tile([PAIR, 1], F32, tag="den")
                nc.vector.tensor_scalar_add(den, m2, 1.0)
                rden = tinyp.tile([PAIR, 1], F32, tag="rden")
                nc.vector.reciprocal(rden, den)
                crow = tinyp.tile([PAIR, E], F32, tag="crow")
                nc.vector.tensor_scalar_mul(crow, gate, rden[:, 0:1])
                nc.tensor.matmul(ct_ps, crow, ident2)
                cT = tinyp.tile([E, PAIR], BF16, tag="cT")
                nc.vector.tensor_copy(cT, ct_ps)

                # group stats: mean gm [32, bb], var -> rstd, into mr bf16
                gm = tinyp.tile([G, PAIR], F32, tag="gm")
                nc.vector.reduce_sum(
                    gm, gs_ps[:, 0:8].rearrange("g (b t) -> g b t", t=NCH), axis=AXX
                )
                mg2 = tinyp.tile([G, PAIR], F32, tag="mg2")
                nc.vector.tensor_tensor(mg2, gm, gm, ALU.mult)
                v = tinyp.tile([G, PAIR], F32, tag="v")
                nc.vector.scalar_tensor_tensor(
                    v, gs_ps[:, 8:10], EPS, mg2, op0=ALU.add, op1=ALU.subtract
                )
                mr = statp.tile([G, PAIR, 2], BF16, tag="mr")
                nc.vector.tensor_copy(mr[:, :, 0], gm)
                # rstd = rsqrt(v): bit-trick seed + 1 Newton step
                yr = tinyp.tile([G, PAIR], F32, tag="yr")
                nc.vector.tensor_tensor(
                    yr[:, :].bitcast(I32),
                    v[:, :].bitcast(I32),
                    one32[:, :].bitcast(I32),
                    ALU.arith_shift_right,
                )
                nc.vector.tensor_tensor(
                    yr[:, :].bitcast(I32),
                    magic32[:, :].bitcast(I32),
                    yr[:, :].bitcast(I32),
                    ALU.subtract,
                )
                t_a = tinyp.tile([G, PAIR], F32, tag="t_a")
                t_b = tinyp.tile([G, PAIR], F32, tag="t_b")
                nc.vector.tensor_tensor(t_a, yr, yr, ALU.mult)
                nc.vector.tensor_tensor(t_b, t_a, v, ALU.mult)
                nc.vector.tensor_scalar(
                    t_a, t_b, -0.5, 1.5, op0=ALU.mult, op1=ALU.add
                )
                nc.vector.tensor_tensor(mr[:, :, 1], yr, t_a, ALU.mult)

                # broadcast group stats to channel partitions; mix experts
                bc = ps_bc[:, ip, :, :]
                nc.tensor.matmul(bc, bmask, mr[:, :, :])
                fu = ps_fu[:, ip, :, :, :]
                for t in range(NCH):
                    nc.tensor.matmul(
                        fu[:, 0, t, :], cb[0:E, P + t * P : P + (t + 1) * P], cT
                    )
                    nc.tensor.matmul(
                        fu[:, 1, t, :], cb[0:E, 640 + t * P : 640 + (t + 1) * P], cT
                    )

                # A = fused_w' * rstd ; B = fused_b' - mean*A   (rstd/mean
                # are per-partition scalars here: group == partition quad)
                bcs = tinyp.tile([P, PAIR, 2], F32, tag="bcs")
                nc.vector.tensor_copy(bcs, bc)
                At = tinyp.tile([P, NCH, PAIR], F32, tag="At")
                t3 = tinyp.tile([P, NCH, PAIR], F32, tag="t3")
                for bb in range(PAIR):
                    nc.vector.tensor_scalar_mul(
                        At[:, :, bb], fu[:, 0, :, bb], bcs[:, bb, 1:2]
                    )
                    nc.vector.tensor_scalar_mul(
                        t3[:, :, bb], At[:, :, bb], bcs[:, bb, 0:1]
                    )
                Bt = tinyp.tile([P, NCH, PAIR], F32, tag="Bt")
                nc.vector.tensor_tensor(Bt, fu[:, 1, :, :], t3, ALU.subtract)

                # pass2: chunks 0,1 on DVE (bf16 fast mode), 2,3 on ACT
                for bb in range(PAIR):
                    b = ip * PAIR + bb
                    y_t = yp.tile([P, NCH, HWD], BF16, tag="y")
                    for j in range(2):
                        nc.vector.tensor_scalar(
                            y_t[:, j, :],
                            xts[bb][:, j, :],
                            At[:, j, bb : bb + 1],
                            Bt[:, j, bb : bb + 1],
                            op0=ALU.mult,
                            op1=ALU.add,
                        )
                    for j in range(2, NCH):
                        nc.scalar.activation(
                            y_t[:, j, :],
                            xts[bb][:, j, :],
                            ACTF.Identity,
                            bias=Bt[:, j, bb : bb + 1],
                            scale=At[:, j, bb : bb + 1],
                        )
                    yv = y_d[b].rearrange("(p t) f -> p t f", p=P)
                    if ip == npair - 1:
                        # last pair: split halves across both HWDGE rings
                        nc.sync.dma_start(out=yv[:, 0:2, :], in_=y_t[:, 0:2, :])
                        nc.scalar.dma_start(out=yv[:, 2:4, :], in_=y_t[:, 2:4, :])
                    else:
                        nc.sync.dma_start(out=yv, in_=y_t)

            # monolithic emission: the scheduler uses emission order as
            # priority, so pair p's chain/pass2/stores must outrank pair
            # p+1's bulk stats work
            for ip in range(npair):
                stage2(ip, stage1(ip))
    nc.finalize()
    return nc


def pack_consts(
    experts_weight, experts_bias, shared_weight, shared_bias, router_w, router_b
):
    import ml_dtypes

    ca = np.zeros((P, CA_W), np.float32)
    # routerT: ca[p, 8t+e] = router_w[e, 4p+t] / HWD
    rw = (router_w / HWD).reshape(E, P, NCH)
    ca[:, 0:32] = np.transpose(rw, (1, 2, 0)).reshape(P, 32)
    pidx = np.arange(P)
    ca[:, 32:64] = GSCALE * (pidx[:, None] // NCH == np.arange(G)[None, :])
    ca[0:PAIR, 64:72] = router_b[None, :]
    ca[0:PAIR, 72:74] = np.eye(PAIR, dtype=np.float32)

    cb = np.zeros((G, CB_W), np.float32)
    cb[:, 0:P] = (np.arange(G)[:, None] == pidx[None, :] // NCH).astype(np.float32)
    # sum(coeff) == 1, so fold the shared affine into every expert row
    ew = (experts_weight + shared_weight[None, :]).reshape(E, P, NCH)
    eb = (experts_bias + shared_bias[None, :]).reshape(E, P, NCH)
    cb[0:E, P : P + C] = np.transpose(ew, (0, 2, 1)).reshape(E, C)
    cb[0:E, P + C : P + 2 * C] = np.transpose(eb, (0, 2, 1)).reshape(E, C)
    return ca, cb.astype(ml_dtypes.bfloat16)


_NC_CACHE: dict[int, bass.Bass] = {}


def _get_nc(n_b: int) -> bass.Bass:
    if n_b not in _NC_CACHE:
        _NC_CACHE[n_b] = build(n_b)
    return _NC_CACHE[n_b]


def run(
    x,
    experts_weight,
    experts_bias,
    shared_weight,
    shared_bias,
    router_w,
    router_b,
    trace: bool = False,
    tmpdir=None,
):
    x = np.ascontiguousarray(np.asarray(x, np.float32)).reshape(B, C, HWD)
    ca, cb = pack_consts(
        np.asarray(experts_weight, np.float32),
        np.asarray(experts_bias, np.float32),
        np.asarray(shared_weight, np.float32),
        np.asarray(shared_bias, np.float32),
        np.asarray(router_w, np.float32),
        np.asarray(router_b, np.float32),
    )
    nc = _get_nc(BPC)
    in_maps = [
        {"x": x[i * BPC : (i + 1) * BPC], "ca": ca, "cb": cb} for i in range(NCORES)
    ]
    res = run_bass_kernel_spmd(
        nc, in_maps, list(range(NCORES)), trace=trace, tmpdir=tmpdir
    )
    y = np.concatenate(
        [res.results[i]["y"].astype(np.float32) for i in range(NCORES)], axis=0
    )
    return y.reshape(B, C, 32, 32), res


def kernel(**inputs) -> np.ndarray:
    y, _ = run(**inputs)
    return y


# revision 21
# speedup vs baseline: 1.1238x; 1.1238x over previous
"""MoE-routed group-norm kernel for Trainium2 (Bass/Tile), 8-core SPMD.

Problem (hardcoded shapes):
  x: [64, 512, 32, 32] f32
  experts_weight/bias: [8, 512], shared_weight/bias: [512]
  router_w: [8, 512], router_b: [8]

  flat = x.mean((2,3)); logits = flat @ router_w.T + router_b
  prob = softmax(logits); top-2 -> coeff = vals / sum(vals)
  fused_w = sum_k coeff_k * experts_weight[idx_k] + shared_weight (bias likewise)
  group-norm over G=32 groups of 16 channels, then y = x_norm * fused_w + fused_b

Strategy: data-parallel over batch, 8 samples per core.

HBM-traffic decisions (this problem is memory-bound):
  * y is stored as bf16 and widened to f32 on the host: 24 MiB/core
    instead of 32 MiB. x is cast f32->bf16 during the load DMA (SWDGE), so
    HBM reads stay f32 but all on-chip passes run 16-bit.
  * channel->partition map is c = 4p + t (NOT c = 128t + p): each
    partition's slice of a sample is CONTIGUOUS in DRAM (16 KiB in, 8 KiB
    out), which gives large DMA descriptors on both directions.
  * all 8 x tiles stay resident in SBUF and every load is pre-issued up
    front on the gpsimd (SWDGE) queue, so the load stream never waits on
    buffer recycling; stores issue from the sync HWDGE ring as each
    sample's pass2 completes.

With c = 4p + t, group(c) = c//16 = p//4: a group never spans chunks, so
group stats are per-partition-quad only:
  s1 per (channel) = two 2-chunk DVE X-reduces per sample
  s2 per partition = one ACT Square+accum_out per sample (sums all 4096)
  group sums = one PE matmul vs gmask32 [128,32] (pre-scaled by 1/16384)
  pass2 y = A*x + B: chunks 0,1 DVE tensor_scalar (bf16, fast mode),
    chunks 2,3 ACT Identity (scale/bias APs)

Routing is the known-good [2,E] pair-batched form: top-1 exp is exactly
1.0 so the softmax denominator cancels in coeff = vals/sum(vals); ACT's
table stays pinned to exp_and_others. Since sum(coeff)=1, shared
weight/bias are folded into the expert tables on the host. rstd uses the
bit-trick seed + one Newton step on DVE (~0.2% rel err, fine at bf16
output precision). Expert mixing / broadcasts are small PE matmuls in
bf16 (fp32 PE matmuls cost 2 instructions each; routing-relevant s1/logits
stay f32 so top-2 selection matches the reference bit-for-bit). PSUM and
ACT-written tiles use static per-pair regions.
"""

import numpy as np

import concourse.bacc as bacc
import concourse.bass as bass
import concourse.tile as tile
from concourse import mybir
from concourse.bass_utils import run_bass_kernel_spmd

F32 = mybir.dt.float32
BF16 = mybir.dt.bfloat16
I32 = mybir.dt.int32
ALU = mybir.AluOpType
ACTF = mybir.ActivationFunctionType
AXX = mybir.AxisListType.X

P = 128            # SBUF partitions
B, C, HWD = 64, 512, 1024
E, G = 8, 32
EPS = 1e-5
NCORES = 8
BPC = B // NCORES  # samples per core
NCH = C // P       # 4 channel chunks per sample (t axis; c = 4p + t)
CPG = C // G       # 16 channels per group
PAIR = 2
RSQRT_MAGIC = 0x5F3759DF
GSCALE = 1.0 / (CPG * HWD)

# cA (f32) layout [128, 74]:
#   0:32  routerT  (ca[p, 8t+e] = router_w[e, 4p+t] / 1024)
#   32:64 gmask32  (ca[p, 32+g] = (p//4 == g) / 16384)
#   64:72 rb2 (rows 0:2) | 72:74 ident2 (rows 0:2)
CA_W = 74
# cB (bf16) layout [32, 1152]:
#   0:128 bmask32 (cb[g, p] = (p//4 == g))
#   rows 0:8 only -- 128:640 ew' (cb[e, 128+128t+p] = ew'[e, 4p+t]) | 640:1152 eb'
CB_W = 1152


def build(n_b: int = BPC) -> bass.Bass:
    assert n_b % PAIR == 0
    npair = n_b // PAIR
    nc = bacc.Bacc()
    x_d = nc.declare_dram_parameter("x", [n_b, C, HWD], F32, isOutput=False)
    ca_d = nc.declare_dram_parameter("ca", [P, CA_W], F32, isOutput=False)
    cb_d = nc.declare_dram_parameter("cb", [G, CB_W], BF16, isOutput=False)
    y_d = nc.declare_dram_parameter("y", [n_b, C, HWD], BF16, isOutput=True)

    with tile.TileContext(nc) as tc:
        with (
            tc.tile_pool(name="consts", bufs=1) as consts,
            tc.tile_pool(name="xp", bufs=n_b) as xp,
            tc.tile_pool(name="yp", bufs=n_b) as yp,
            tc.tile_pool(name="scr", bufs=2) as scrp,
            tc.tile_pool(name="statp", bufs=4) as statp,
            tc.tile_pool(name="tinyp", bufs=4) as tinyp,
            tc.tile_pool(name="ps_static", bufs=1, space="PSUM") as pstat,
        ):
            # consts staged through a DVE copy so PE inputs have DVE provenance
            ca_st = consts.tile([P, CA_W], F32)
            nc.sync.dma_start(out=ca_st, in_=ca_d[:, :])
            cb_st = consts.tile([G, CB_W], BF16)
            nc.sync.dma_start(out=cb_st, in_=cb_d[:, :])
            ca = consts.tile([P, CA_W], F32)
            nc.vector.tensor_copy(ca, ca_st)
            cb = consts.tile([G, CB_W], BF16)
            nc.vector.tensor_copy(cb, cb_st)
            magic32 = consts.tile([G, PAIR], F32)
            nc.vector.memset(magic32[:, :].bitcast(I32), RSQRT_MAGIC)
            one32 = consts.tile([G, PAIR], F32)
            nc.vector.memset(one32[:, :].bitcast(I32), 1)

            gmask = ca[:, 32:64]
            rb2 = ca[0:PAIR, 64:72]
            ident2 = ca[0:PAIR, 72:74]
            bmask = cb[:, 0:P]

            # all 8 x tiles resident; every load pre-issued on the SWDGE
            # queue (f32 -> bf16 cast during DMA), two halves per sample
            xts_all = []
            for b in range(n_b):
                x_t = xp.tile([P, NCH, HWD], BF16, tag="x")
                xts_all.append(x_t)
                xv = x_d[b].rearrange("(p t) f -> p t f", p=P)
                if b == 0:
                    for j4 in range(NCH):
                        nc.gpsimd.dma_start(
                            out=x_t[:, j4 : j4 + 1, :], in_=xv[:, j4 : j4 + 1, :]
                        )
                else:
                    nc.gpsimd.dma_start(out=x_t[:, 0:2, :], in_=xv[:, 0:2, :])
                    nc.gpsimd.dma_start(out=x_t[:, 2:4, :], in_=xv[:, 2:4, :])

            # static per-pair PSUM regions (never reused -> no PSUM WAW deps)
            ps_sm = pstat.tile([P, npair, 24], F32, tag="sm")
            ps_fu = pstat.tile([P, npair, 2, NCH, PAIR], F32, tag="fu")
            ps_bc = pstat.tile([P, npair, PAIR, 2], F32, tag="bc")
            erow_all = consts.tile([PAIR, npair, E], F32)

            def stage1(ip):
                xts = [xts_all[ip * PAIR], xts_all[ip * PAIR + 1]]
                # s12 [P, 12]: cols 0:8 s1 per (bb, t), 8:12 s2 halves
                s12 = statp.tile([P, 12], F32, tag="s12")
                s1v = s12[:, 0:8].rearrange("p (b t) -> p b t", t=NCH)
                lg_ps = ps_sm[0:PAIR, ip, 12:20]      # logits [2, 8]

                for bb in range(PAIR):
                    # two 2-chunk reduces: smaller granules let the
                    # scheduler slot chain ops between them
                    nc.vector.reduce_sum(
                        s1v[:, bb, 0:2], xts[bb][:, 0:2, :], axis=AXX
                    )
                    nc.vector.reduce_sum(
                        s1v[:, bb, 2:4], xts[bb][:, 2:4, :], axis=AXX
                    )
                    for h in range(2):
                        sq = scrp.tile([P, 2, HWD], BF16, tag="sq")
                        nc.scalar.activation(
                            sq,
                            xts[bb][:, 2 * h : 2 * h + 2, :],
                            ACTF.Square,
                            bias=0.0,
                            scale=1.0,
                            accum_out=s12[:, 8 + 2 * bb + h : 9 + 2 * bb + h],
                        )

                # logits[s, e] = sum_c s1[c, s]/1024 * router_w[e, c]
                for t in range(NCH):
                    nc.tensor.matmul(
                        lg_ps,
                        s1v[:, :, t],
                        ca[:, t * 8 : (t + 1) * 8],
                        start=(t == 0),
                        stop=(t == NCH - 1),
                    )
                # group sums of (s1 | s2), pre-scaled by 1/16384 via gmask
                gs_ps = ps_sm[0:G, ip, 0:12]
                nc.tensor.matmul(gs_ps, gmask, s12[:, :])
                return xts

            def stage2(ip, xts):
                gs_ps = ps_sm[0:G, ip, 0:12]
                lg_ps = ps_sm[0:PAIR, ip, 12:20]
                ct_ps = ps_sm[0:E, ip, 20:22]

                # routing, pair-batched in [2, E] partition layout
                lrow = tinyp.tile([PAIR, E], F32, tag="lrow")
                nc.vector.tensor_tensor(lrow, lg_ps, rb2, ALU.add)
                nmax = tinyp.tile([PAIR, 1], F32, tag="nmax")
                nc.vector.reduce_max(nmax, lrow, axis=AXX, negate=True)
                erow = erow_all[:, ip, :]
                nc.scalar.activation(erow, lrow, ACTF.Exp, bias=nmax, scale=1.0)
                qrow = tinyp.tile([PAIR, E], F32, tag="qrow")
                nc.vector.scalar_tensor_tensor(
                    qrow, erow, 1.0, erow, op0=ALU.is_lt, op1=ALU.mult
                )
                m2 = tinyp.tile([PAIR, 1], F32, tag="m2")
                nc.vector.reduce_max(m2, qrow, axis=AXX)
                gate = tinyp.tile([PAIR, E], F32, tag="gate")
                nc.vector.scalar_tensor_tensor(
                    gate, erow, m2[:, 0:1], erow, op0=ALU.is_ge, op1=ALU.mult
                )
                den = tinyp.tile([PAIR, 1], F32, tag="den")
                nc.vector.tensor_scalar_add(den, m2, 1.0)
                rden = tinyp.tile([PAIR, 1], F32, tag="rden")
                nc.vector.reciprocal(rden, den)
                crow = tinyp.tile([PAIR, E], F32, tag="crow")
                nc.vector.tensor_scalar_mul(crow, gate, rden[:, 0:1])
                nc.tensor.matmul(ct_ps, crow, ident2)
                cT = tinyp.tile([E, PAIR], BF16, tag="cT")
                nc.vector.tensor_copy(cT, ct_ps)

                # group stats: mean gm [32, bb], var -> rstd, into mr bf16
                gm = tinyp.tile([G, PAIR], F32, tag="gm")
                nc.vector.reduce_sum(
                    gm, gs_ps[:, 0:8].rearrange("g (b t) -> g b t", t=NCH), axis=AXX
                )
                mg2 = tinyp.tile([G, PAIR], F32, tag="mg2")
                nc.vector.tensor_tensor(mg2, gm, gm, ALU.mult)
                s2s = tinyp.tile([G, PAIR], F32, tag="s2s")
                nc.vector.reduce_sum(
                    s2s, gs_ps[:, 8:12].rearrange("g (b h) -> g b h", h=2), axis=AXX
                )
                v = tinyp.tile([G, PAIR], F32, tag="v")
                nc.vector.scalar_tensor_tensor(
                    v, s2s, EPS, mg2, op0=ALU.add, op1=ALU.subtract
                )
                mr = statp.tile([G, PAIR, 2], BF16, tag="mr")
                nc.vector.tensor_copy(mr[:, :, 0], gm)
                # rstd = rsqrt(v): bit-trick seed + 1 Newton step
                yr = tinyp.tile([G, PAIR], F32, tag="yr")
                nc.vector.tensor_tensor(
                    yr[:, :].bitcast(I32),
                    v[:, :].bitcast(I32),
                    one32[:, :].bitcast(I32),
                    ALU.arith_shift_right,
                )
                nc.vector.tensor_tensor(
                    yr[:, :].bitcast(I32),
                    magic32[:, :].bitcast(I32),
                    yr[:, :].bitcast(I32),
                    ALU.subtract,
                )
                t_a = tinyp.tile([G, PAIR], F32, tag="t_a")
                t_b = tinyp.tile([G, PAIR], F32, tag="t_b")
                nc.vector.tensor_tensor(t_a, yr, yr, ALU.mult)
                nc.vector.tensor_tensor(t_b, t_a, v, ALU.mult)
                nc.vector.tensor_scalar(
                    t_a, t_b, -0.5, 1.5, op0=ALU.mult, op1=ALU.add
                )
                nc.vector.tensor_tensor(mr[:, :, 1], yr, t_a, ALU.mult)

                # broadcast group stats to channel partitions; mix experts
                bc = ps_bc[:, ip, :, :]
                nc.tensor.matmul(bc, bmask, mr[:, :, :])
                fu = ps_fu[:, ip, :, :, :]
                for t in range(NCH):
                    nc.tensor.matmul(
                        fu[:, 0, t, :], cb[0:E, P + t * P : P + (t + 1) * P], cT
                    )
                    nc.tensor.matmul(
                        fu[:, 1, t, :], cb[0:E, 640 + t * P : 640 + (t + 1) * P], cT
                    )

                # A = fused_w' * rstd ; B = fused_b' - mean*A   (rstd/mean
                # are per-partition scalars here: group == partition quad)
                bcs = tinyp.tile([P, PAIR, 2], F32, tag="bcs")
                nc.vector.tensor_copy(bcs, bc)
                At = tinyp.tile([P, NCH, PAIR], F32, tag="At")
                t3 = tinyp.tile([P, NCH, PAIR], F32, tag="t3")
                for bb in range(PAIR):
                    nc.vector.tensor_scalar_mul(
                        At[:, :, bb], fu[:, 0, :, bb], bcs[:, bb, 1:2]
                    )
                    nc.vector.tensor_scalar_mul(
                        t3[:, :, bb], At[:, :, bb], bcs[:, bb, 0:1]
                    )
                Bt = tinyp.tile([P, NCH, PAIR], F32, tag="Bt")
                nc.vector.tensor_tensor(Bt, fu[:, 1, :, :], t3, ALU.subtract)

                # pass2: chunks 0,1 on DVE (bf16 fast mode), 2,3 on ACT
                for bb in range(PAIR):
                    b = ip * PAIR + bb
                    y_t = yp.tile([P, NCH, HWD], BF16, tag="y")
                    for j in range(2):
                        nc.vector.tensor_scalar(
                            y_t[:, j, :],
                            xts[bb][:, j, :],
                            At[:, j, bb : bb + 1],
                            Bt[:, j, bb : bb + 1],
                            op0=ALU.mult,
                            op1=ALU.add,
                        )
                    for j in range(2, NCH):
                        nc.scalar.activation(
                            y_t[:, j, :],
                            xts[bb][:, j, :],
                            ACTF.Identity,
                            bias=Bt[:, j, bb : bb + 1],
                            scale=At[:, j, bb : bb + 1],
                        )
                    yv = y_d[b].rearrange("(p t) f -> p t f", p=P)
                    # half-stores: the DVE-written half (chunks 0,1) leaves
                    # as soon as it is ready, without waiting for ACT
                    nc.sync.dma_start(out=yv[:, 0:2, :], in_=y_t[:, 0:2, :])
                    if ip == npair - 1:
                        nc.scalar.dma_start(out=yv[:, 2:4, :], in_=y_t[:, 2:4, :])
                    else:
                        nc.sync.dma_start(out=yv[:, 2:4, :], in_=y_t[:, 2:4, :])

            # monolithic emission: the scheduler uses emission order as
            # priority, so pair p's chain/pass2/stores must outrank pair
            # p+1's bulk stats work
            for ip in range(npair):
                stage2(ip, stage1(ip))
    nc.finalize()
    return nc


def pack_consts(
    experts_weight, experts_bias, shared_weight, shared_bias, router_w, router_b
):
    import ml_dtypes

    ca = np.zeros((P, CA_W), np.float32)
    # routerT: ca[p, 8t+e] = router_w[e, 4p+t] / HWD
    rw = (router_w / HWD).reshape(E, P, NCH)
    ca[:, 0:32] = np.transpose(rw, (1, 2, 0)).reshape(P, 32)
    pidx = np.arange(P)
    ca[:, 32:64] = GSCALE * (pidx[:, None] // NCH == np.arange(G)[None, :])
    ca[0:PAIR, 64:72] = router_b[None, :]
    ca[0:PAIR, 72:74] = np.eye(PAIR, dtype=np.float32)

    cb = np.zeros((G, CB_W), np.float32)
    cb[:, 0:P] = (np.arange(G)[:, None] == pidx[None, :] // NCH).astype(np.float32)
    # sum(coeff) == 1, so fold the shared affine into every expert row
    ew = (experts_weight + shared_weight[None, :]).reshape(E, P, NCH)
    eb = (experts_bias + shared_bias[None, :]).reshape(E, P, NCH)
    cb[0:E, P : P + C] = np.transpose(ew, (0, 2, 1)).reshape(E, C)
    cb[0:E, P + C : P + 2 * C] = np.transpose(eb, (0, 2, 1)).reshape(E, C)
    return ca, cb.astype(ml_dtypes.bfloat16)


_NC_CACHE: dict[int, bass.Bass] = {}


def _get_nc(n_b: int) -> bass.Bass:
    if n_b not in _NC_CACHE:
        _NC_CACHE[n_b] = build(n_b)
    return _NC_CACHE[n_b]


def run(
    x,
    experts_weight,
    experts_bias,
    shared_weight,
    shared_bias,
    router_w,
    router_b,
    trace: bool = False,
    tmpdir=None,
):
    x = np.ascontiguousarray(np.asarray(x, np.float32)).reshape(B, C, HWD)
    ca, cb = pack_consts(
        np.asarray(experts_weight, np.float32),
        np.asarray(experts_bias, np.float32),
        np.asarray(shared_weight, np.float32),
        np.asarray(shared_bias, np.float32),
        np.asarray(router_w, np.float32),
        np.asarray(router_b, np.float32),
    )
    nc = _get_nc(BPC)
    in_maps = [
        {"x": x[i * BPC : (i + 1) * BPC], "ca": ca, "cb": cb} for i in range(NCORES)
    ]
    res = run_bass_kernel_spmd(
        nc, in_maps, list(range(NCORES)), trace=trace, tmpdir=tmpdir
    )
    y = np.concatenate(
        [res.results[i]["y"].astype(np.float32) for i in range(NCORES)], axis=0
    )
    return y.reshape(B, C, 32, 32), res


def kernel(**inputs) -> np.ndarray:
    y, _ = run(**inputs)
    return y
